# revision 32
# baseline (speedup 1.0000x reference)
"""CLUB loss kernel for Trainium2, 8-core data-parallel SPMD.

Math: with flat_x (N,D) [from x (B,D,H,W) -> (B*H*W, D)], v = exp(-p_logvar),
  loss = mean_i[ -0.5*sum_d ((x-mu)^2 - (m2 - 2*mu*m1 + mu^2)) * v ]
       = (-0.5/N) * [ A - 2B - dot(m2, V) + 2*dot(m1, W) ]
where
  A  = sum_{i,d} x^2 v          B  = sum_{i,d} x mu v
  V_d = sum_i v                 W_d = sum_i mu v
  m1 = S1/N, m2 = S2/N,  S1_d = sum_i x,  S2_d = sum_i x^2
All terms are per-core-local partial sums; the tiny (~KB) cross-core
reduction and final dot products happen on host in float64. No collectives.

Layout: everything in d-major (partition = d) so that every reduction above
is a sum over the FREE axis and rides for free on `accum_out` of ops we run
anyway. x streams in natively d-major; mu and logvar are transposed on the
PE (128x128 identity-matmul blocks) into PSUM, and the ACT/DVE consumers
read straight from PSUM, fusing evacuation with compute:
  ACT: v = exp(-lvT)   [PSUM->SBUF]  + accum -> V
  ACT: p = square(x)                 + accum -> S2
  ACT: copy(x)                       + accum -> S1
  DVE: w = muT * v     [PSUM->SBUF]  + accum -> W
  DVE: a = p * v                     + accum -> A-partials
  DVE: b = x * w                     + accum -> B-partials
PE does ONLY the 128 block transposes. No reduction matmuls at all.

Processing unit = (b-block, d-chunk): (128 d) x (1024 i) tiles.
"""

import sys

import numpy as np

for _p in ("/opt/trn_rl_repo",):
    if _p not in sys.path:
        sys.path.append(_p)

B, D, H, W = 16, 512, 32, 32
HW = H * W
N = B * HW
NCORES = 8
BLKB = B // NCORES          # b-blocks per core (2)
ROWS = N // NCORES          # rows per core (2048)
NT = ROWS // 128            # 128-row i-tiles per core (16)
NDC = D // 128              # d chunks (4)
SLAB = 4                    # i-tiles per mu/lv DMA slab
NU = BLKB * NDC             # processing units per core (8)

_prog_cache = {}


def build_program():
    import concourse.bacc as bacc
    import concourse.tile as tile
    from concourse import mybir

    f32 = mybir.dt.float32
    AF = mybir.ActivationFunctionType
    OP = mybir.AluOpType

    nc = bacc.Bacc(
        "TRN2",
        target_bir_lowering=False,
        debug=False,
        enable_asserts=False,
        num_devices=NCORES,
    )

    x_d = nc.dram_tensor("x_s", (BLKB, D, HW), f32, kind="ExternalInput").ap()
    mu_d = nc.dram_tensor("mu_s", (ROWS, D), f32, kind="ExternalInput").ap()
    lv_d = nc.dram_tensor("lv_s", (ROWS, D), f32, kind="ExternalInput").ap()
    id_d = nc.dram_tensor("ident", (128, 128), f32, kind="ExternalInput").ap()

    # o_misc col q*NU+u, quantity q: 0 V, 1 W, 2 S1, 3 S2, 4 A, 5 B
    # unit u = b*NDC+dc; partition p -> d = 128*dc+p
    o_misc = nc.dram_tensor("o_misc", (128, 6 * NU), f32, kind="ExternalOutput").ap()

    with tile.TileContext(nc) as tc:
        with (
            tc.tile_pool(name="const", bufs=1) as constp,
            tc.tile_pool(name="xnat", bufs=1) as xp,
            tc.tile_pool(name="slab", bufs=3) as slp,
            tc.tile_pool(name="stream", bufs=2) as sp,
            tc.tile_pool(name="accum", bufs=1) as accp,
            tc.tile_pool(name="psum", bufs=2, space="PSUM") as pp,
        ):
            ident = constp.tile([128, 128], f32)
            nc.sync.dma_start(ident[:], id_d[:])

            # one private accumulator tile per quantity: disjoint tiles keep
            # Tile's dependency tracking from serializing across engines
            acc = [
                accp.tile([128, NU], f32, tag=f"acc{q}", name=f"acc{q}")
                for q in range(6)
            ]

            def load_one(dram, s, tag):
                rows = dram[128 * SLAB * s : 128 * SLAB * (s + 1), :]
                t_ = slp.tile([128, SLAB * D], f32, tag=tag, name=tag)
                nc.gpsimd.dma_start(t_[:], rows.rearrange("(g p) f -> p g f", p=128))
                return t_

            def load_x(b):
                t_ = xp.tile([128, NDC * HW], f32, tag=f"x_{b}", name=f"x_{b}")
                nc.sync.dma_start(
                    t_[:], x_d[b].rearrange("(dc p) hw -> p dc hw", p=128)
                )
                return t_

            # issue order tuned for earliest first-unit compute:
            # lv block-0 first (feeds first transposes + exp), then mu
            # block-0 interleaved with x block-0.
            lv0, lv1 = load_one(lv_d, 0, "lv_sl"), load_one(lv_d, 1, "lv_sl")
            mu0 = load_one(mu_d, 0, "mu_sl")
            x_sb = [load_x(0)]
            mu1 = load_one(mu_d, 1, "mu_sl")
            x_sb.append(load_x(1))
            slabs = {0: (mu0, lv0), 1: (mu1, lv1)}

            for u in range(NU):
                b, dc = divmod(u, NDC)
                if b > 0 and dc == 0:
                    slabs[2 * b] = (
                        load_one(mu_d, 2 * b, "mu_sl"),
                        load_one(lv_d, 2 * b, "lv_sl"),
                    )
                    slabs[2 * b + 1] = (
                        load_one(mu_d, 2 * b + 1, "mu_sl"),
                        load_one(lv_d, 2 * b + 1, "lv_sl"),
                    )

                # transpose this unit's lv and mu blocks: (1024 i x 128 d)
                # -> PSUM (128 d x 1024 i), 8 blocks each, 4 per psum bank
                lvT = pp.tile([128, HW], f32, tag="lvT", name="lvT")
                muT = pp.tile([128, HW], f32, tag="muT", name="muT")
                for blk in range(8):
                    t_i = 8 * b + blk          # global i-tile index
                    sl = slabs[t_i // SLAB]
                    col = D * (t_i % SLAB) + 128 * dc
                    for dst, src in ((lvT, sl[1]), (muT, sl[0])):
                        nc.tensor.matmul(
                            dst[:, 128 * blk : 128 * (blk + 1)],
                            src[:, col : col + 128],
                            ident[:],
                            is_transpose=True,
                            start=(blk % 4 == 0),
                            stop=(blk % 4 == 3),
                        )

                xs = x_sb[b][:, HW * dc : HW * (dc + 1)]

                v_u = sp.tile([128, HW], f32, tag="v", name="v_u")
                nc.scalar.activation(
                    v_u[:], lvT[:], AF.Exp, scale=-1.0,
                    accum_out=acc[0][:, u : u + 1],
                )
                p_u = sp.tile([128, HW], f32, tag="p", name="p_u")
                nc.scalar.activation(
                    p_u[:], xs, AF.Square,
                    accum_out=acc[3][:, u : u + 1],
                )
                s1scr = sp.tile([128, HW], f32, tag="s1scr", name="s1scr")
                nc.scalar.activation(
                    s1scr[:], xs, AF.Copy,
                    accum_out=acc[2][:, u : u + 1],
                )

                w_u = sp.tile([128, HW], f32, tag="w", name="w_u")
                nc.vector.scalar_tensor_tensor(
                    out=w_u[:], in0=muT[:], scalar=1.0, in1=v_u[:],
                    op0=OP.mult, op1=OP.mult,
                    accum_out=acc[1][:, u : u + 1],
                )
                a_u = sp.tile([128, HW], f32, tag="a", name="a_u")
                nc.vector.scalar_tensor_tensor(
                    out=a_u[:], in0=p_u[:], scalar=1.0, in1=v_u[:],
                    op0=OP.mult, op1=OP.mult,
                    accum_out=acc[4][:, u : u + 1],
                )
                b_u = sp.tile([128, HW], f32, tag="b", name="b_u")
                nc.vector.scalar_tensor_tensor(
                    out=b_u[:], in0=w_u[:], scalar=1.0, in1=xs,
                    op0=OP.mult, op1=OP.mult,
                    accum_out=acc[5][:, u : u + 1],
                )

            for q in range(6):
                nc.sync.dma_start(o_misc[:, NU * q : NU * (q + 1)], acc[q][:])

    nc.compile()
    return nc


def get_program():
    if "nc" not in _prog_cache:
        _prog_cache["nc"] = build_program()
    return _prog_cache["nc"]


def make_in_maps(x, p_mu, p_logvar):
    x = np.ascontiguousarray(np.asarray(x, dtype=np.float32)).reshape(B, D, HW)
    p_mu = np.ascontiguousarray(np.asarray(p_mu, dtype=np.float32))
    p_logvar = np.ascontiguousarray(np.asarray(p_logvar, dtype=np.float32))
    in_maps = []
    for c in range(NCORES):
        in_maps.append(
            {
                "x_s": np.ascontiguousarray(x[BLKB * c : BLKB * (c + 1)]),
                "mu_s": np.ascontiguousarray(p_mu[ROWS * c : ROWS * (c + 1)]),
                "lv_s": np.ascontiguousarray(p_logvar[ROWS * c : ROWS * (c + 1)]),
                "ident": np.eye(128, dtype=np.float32),
            }
        )
    return in_maps


def finish_host(results):
    """Combine per-core partials (float64) into the scalar loss."""
    Vv = np.zeros(D)
    Ww = np.zeros(D)
    S2 = np.zeros(D)
    S1 = np.zeros(D)
    A = 0.0
    Bb = 0.0
    for r in results:
        misc = r["o_misc"].astype(np.float64)
        for u in range(NU):
            b, dc = divmod(u, NDC)
            dsl = slice(128 * dc, 128 * (dc + 1))
            Vv[dsl] += misc[:, u]
            Ww[dsl] += misc[:, NU + u]
            S1[dsl] += misc[:, 2 * NU + u]
            S2[dsl] += misc[:, 3 * NU + u]
            A += float(misc[:, 4 * NU + u].sum())
            Bb += float(misc[:, 5 * NU + u].sum())
    m1 = S1 / N
    m2 = S2 / N
    S = A - 2.0 * Bb - float(np.dot(m2, Vv)) + 2.0 * float(np.dot(m1, Ww))
    return np.float32(-0.5 / N * S)


def run_on_device(x, p_mu, p_logvar, trace=False, **kw):
    from concourse import bass_utils

    nc = get_program()
    in_maps = make_in_maps(x, p_mu, p_logvar)
    return bass_utils.run_bass_kernel_spmd(
        nc, in_maps, list(range(NCORES)), trace=trace, **kw
    )


def kernel(x, p_mu, p_logvar):
    res = run_on_device(x, p_mu, p_logvar)
    return finish_host(res.results)


# revision 34
# speedup vs baseline: 1.0510x; 1.0510x over previous
"""CLUB loss kernel for Trainium2, 8-core data-parallel SPMD.

Math: with flat_x (N,D) [from x (B,D,H,W) -> (B*H*W, D)], v = exp(-p_logvar),
  loss = mean_i[ -0.5*sum_d ((x-mu)^2 - (m2 - 2*mu*m1 + mu^2)) * v ]
       = (-0.5/N) * [ A - 2B - dot(m2, V) + 2*dot(m1, W) ]
where
  A  = sum_{i,d} x^2 v          B  = sum_{i,d} x mu v
  V_d = sum_i v                 W_d = sum_i mu v
  m1 = S1/N, m2 = S2/N,  S1_d = sum_i x,  S2_d = sum_i x^2
All terms are per-core-local partial sums; the tiny (~KB) cross-core
reduction and final dot products happen on host in float64. No collectives.

Layout: everything in d-major (partition = d) so that every reduction above
is a sum over the FREE axis and rides for free on `accum_out` of ops we run
anyway. x streams in natively d-major; mu and logvar are transposed on the
PE (128x128 identity-matmul blocks) into PSUM, and the ACT/DVE consumers
read straight from PSUM, fusing evacuation with compute:
  ACT: v = exp(-lvT)   [PSUM->SBUF]  + accum -> V
  ACT: p = square(x)                 + accum -> S2
  ACT: copy(x)                       + accum -> S1
  DVE: w = muT * v     [PSUM->SBUF]  + accum -> W
  DVE: a = p * v                     + accum -> A-partials
  DVE: b = x * w                     + accum -> B-partials
PE does ONLY the 128 block transposes. No reduction matmuls at all.

Processing unit = (b-block, d-chunk): (128 d) x (1024 i) tiles.
"""

import sys

import numpy as np

for _p in ("/opt/trn_rl_repo",):
    if _p not in sys.path:
        sys.path.append(_p)

B, D, H, W = 16, 512, 32, 32
HW = H * W
N = B * HW
NCORES = 8
BLKB = B // NCORES          # b-blocks per core (2)
ROWS = N // NCORES          # rows per core (2048)
NT = ROWS // 128            # 128-row i-tiles per core (16)
NDC = D // 128              # d chunks (4)
SLAB = 4                    # i-tiles per mu/lv DMA slab
NU = BLKB * NDC             # processing units per core (8)

_prog_cache = {}


def build_program():
    import concourse.bacc as bacc
    import concourse.tile as tile
    from concourse import mybir

    f32 = mybir.dt.float32
    AF = mybir.ActivationFunctionType
    OP = mybir.AluOpType

    nc = bacc.Bacc(
        "TRN2",
        target_bir_lowering=False,
        debug=False,
        enable_asserts=False,
        num_devices=NCORES,
    )

    x_d = nc.dram_tensor("x_s", (BLKB, D, HW), f32, kind="ExternalInput").ap()
    mu_d = nc.dram_tensor("mu_s", (ROWS, D), f32, kind="ExternalInput").ap()
    lv_d = nc.dram_tensor("lv_s", (ROWS, D), f32, kind="ExternalInput").ap()
    id_d = nc.dram_tensor("ident", (128, 128), f32, kind="ExternalInput").ap()

    # o_misc col q*NU+u, quantity q: 0 V, 1 W, 2 S1, 3 S2, 4 A, 5 B
    # unit u = b*NDC+dc; partition p -> d = 128*dc+p
    o_misc = nc.dram_tensor("o_misc", (128, 6 * NU), f32, kind="ExternalOutput").ap()

    with tile.TileContext(nc) as tc:
        with (
            tc.tile_pool(name="const", bufs=1) as constp,
            tc.tile_pool(name="xnat", bufs=1) as xp,
            tc.tile_pool(name="slab", bufs=3) as slp,
            tc.tile_pool(name="stream", bufs=4) as sp,
            tc.tile_pool(name="accum", bufs=1) as accp,
            tc.tile_pool(name="psum", bufs=2, space="PSUM") as pp,
        ):
            ident = constp.tile([128, 128], f32)
            nc.sync.dma_start(ident[:], id_d[:])

            # one private accumulator tile per quantity: disjoint tiles keep
            # Tile's dependency tracking from serializing across engines
            acc = [
                accp.tile([128, NU], f32, tag=f"acc{q}", name=f"acc{q}")
                for q in range(6)
            ]

            def load_one(dram, s, tag):
                rows = dram[128 * SLAB * s : 128 * SLAB * (s + 1), :]
                t_ = slp.tile([128, SLAB * D], f32, tag=tag, name=tag)
                nc.gpsimd.dma_start(t_[:], rows.rearrange("(g p) f -> p g f", p=128))
                return t_

            def load_x(b):
                t_ = xp.tile([128, NDC * HW], f32, tag=f"x_{b}", name=f"x_{b}")
                nc.sync.dma_start(
                    t_[:], x_d[b].rearrange("(dc p) hw -> p dc hw", p=128)
                )
                return t_

            # issue order tuned for earliest first-unit compute:
            # lv block-0 first (feeds first transposes + exp), then mu
            # block-0 interleaved with x block-0.
            lv0, lv1 = load_one(lv_d, 0, "lv_sl"), load_one(lv_d, 1, "lv_sl")
            mu0 = load_one(mu_d, 0, "mu_sl")
            x_sb = [load_x(0)]
            mu1 = load_one(mu_d, 1, "mu_sl")
            x_sb.append(load_x(1))
            slabs = {0: (mu0, lv0), 1: (mu1, lv1)}

            for u in range(NU):
                b, dc = divmod(u, NDC)
                if b > 0 and dc == 0:
                    slabs[2 * b] = (
                        load_one(mu_d, 2 * b, "mu_sl"),
                        load_one(lv_d, 2 * b, "lv_sl"),
                    )
                    slabs[2 * b + 1] = (
                        load_one(mu_d, 2 * b + 1, "mu_sl"),
                        load_one(lv_d, 2 * b + 1, "lv_sl"),
                    )

                # transpose this unit's lv and mu blocks: (1024 i x 128 d)
                # -> PSUM (128 d x 1024 i), 8 blocks each, 4 per psum bank
                lvT = pp.tile([128, HW], f32, tag="lvT", name="lvT")
                muT = pp.tile([128, HW], f32, tag="muT", name="muT")
                for blk in range(8):
                    t_i = 8 * b + blk          # global i-tile index
                    sl = slabs[t_i // SLAB]
                    col = D * (t_i % SLAB) + 128 * dc
                    for dst, src in ((lvT, sl[1]), (muT, sl[0])):
                        nc.tensor.matmul(
                            dst[:, 128 * blk : 128 * (blk + 1)],
                            src[:, col : col + 128],
                            ident[:],
                            is_transpose=True,
                            start=(blk % 4 == 0),
                            stop=(blk % 4 == 3),
                        )

                xs = x_sb[b][:, HW * dc : HW * (dc + 1)]

                v_u = sp.tile([128, HW], f32, tag="v", name="v_u")
                nc.scalar.activation(
                    v_u[:], lvT[:], AF.Exp, scale=-1.0,
                    accum_out=acc[0][:, u : u + 1],
                )
                p_u = sp.tile([128, HW], f32, tag="p", name="p_u")
                nc.scalar.activation(
                    p_u[:], xs, AF.Square,
                    accum_out=acc[3][:, u : u + 1],
                )
                # S1: split between ACT (Copy+accum) and DVE (tensor_reduce)
                # to balance engine load
                if u % 2 == 0:
                    s1scr = sp.tile([128, HW], f32, tag="s1scr", name="s1scr")
                    nc.scalar.activation(
                        s1scr[:], xs, AF.Copy,
                        accum_out=acc[2][:, u : u + 1],
                    )
                else:
                    nc.vector.tensor_reduce(
                        acc[2][:, u : u + 1], xs, mybir.AxisListType.X, OP.add
                    )

                w_u = sp.tile([128, HW], f32, tag="w", name="w_u")
                nc.vector.scalar_tensor_tensor(
                    out=w_u[:], in0=muT[:], scalar=1.0, in1=v_u[:],
                    op0=OP.mult, op1=OP.mult,
                    accum_out=acc[1][:, u : u + 1],
                )
                a_u = sp.tile([128, HW], f32, tag="a", name="a_u")
                nc.vector.scalar_tensor_tensor(
                    out=a_u[:], in0=p_u[:], scalar=1.0, in1=v_u[:],
                    op0=OP.mult, op1=OP.mult,
                    accum_out=acc[4][:, u : u + 1],
                )
                b_u = sp.tile([128, HW], f32, tag="b", name="b_u")
                nc.vector.scalar_tensor_tensor(
                    out=b_u[:], in0=w_u[:], scalar=1.0, in1=xs,
                    op0=OP.mult, op1=OP.mult,
                    accum_out=acc[5][:, u : u + 1],
                )

            for q in range(6):
                nc.sync.dma_start(o_misc[:, NU * q : NU * (q + 1)], acc[q][:])

    nc.compile()
    return nc


def get_program():
    if "nc" not in _prog_cache:
        _prog_cache["nc"] = build_program()
    return _prog_cache["nc"]


def make_in_maps(x, p_mu, p_logvar):
    x = np.ascontiguousarray(np.asarray(x, dtype=np.float32)).reshape(B, D, HW)
    p_mu = np.ascontiguousarray(np.asarray(p_mu, dtype=np.float32))
    p_logvar = np.ascontiguousarray(np.asarray(p_logvar, dtype=np.float32))
    in_maps = []
    for c in range(NCORES):
        in_maps.append(
            {
                "x_s": np.ascontiguousarray(x[BLKB * c : BLKB * (c + 1)]),
                "mu_s": np.ascontiguousarray(p_mu[ROWS * c : ROWS * (c + 1)]),
                "lv_s": np.ascontiguousarray(p_logvar[ROWS * c : ROWS * (c + 1)]),
                "ident": np.eye(128, dtype=np.float32),
            }
        )
    return in_maps


def finish_host(results):
    """Combine per-core partials (float64) into the scalar loss."""
    Vv = np.zeros(D)
    Ww = np.zeros(D)
    S2 = np.zeros(D)
    S1 = np.zeros(D)
    A = 0.0
    Bb = 0.0
    for r in results:
        misc = r["o_misc"].astype(np.float64)
        for u in range(NU):
            b, dc = divmod(u, NDC)
            dsl = slice(128 * dc, 128 * (dc + 1))
            Vv[dsl] += misc[:, u]
            Ww[dsl] += misc[:, NU + u]
            S1[dsl] += misc[:, 2 * NU + u]
            S2[dsl] += misc[:, 3 * NU + u]
            A += float(misc[:, 4 * NU + u].sum())
            Bb += float(misc[:, 5 * NU + u].sum())
    m1 = S1 / N
    m2 = S2 / N
    S = A - 2.0 * Bb - float(np.dot(m2, Vv)) + 2.0 * float(np.dot(m1, Ww))
    return np.float32(-0.5 / N * S)


def run_on_device(x, p_mu, p_logvar, trace=False, **kw):
    from concourse import bass_utils

    nc = get_program()
    in_maps = make_in_maps(x, p_mu, p_logvar)
    return bass_utils.run_bass_kernel_spmd(
        nc, in_maps, list(range(NCORES)), trace=trace, **kw
    )


def kernel(x, p_mu, p_logvar):
    res = run_on_device(x, p_mu, p_logvar)
    return finish_host(res.results)


# revision 36
# speedup vs baseline: 1.1083x; 1.0545x over previous
"""CLUB loss kernel for Trainium2, 8-core data-parallel SPMD.

Math: with flat_x (N,D) [from x (B,D,H,W) -> (B*H*W, D)], v = exp(-p_logvar),
  loss = mean_i[ -0.5*sum_d ((x-mu)^2 - (m2 - 2*mu*m1 + mu^2)) * v ]
       = (-0.5/N) * [ A - 2B - dot(m2, V) + 2*dot(m1, W) ]
where
  A  = sum_{i,d} x^2 v          B  = sum_{i,d} x mu v
  V_d = sum_i v                 W_d = sum_i mu v
  m1 = S1/N, m2 = S2/N,  S1_d = sum_i x,  S2_d = sum_i x^2
All terms are per-core-local partial sums; the tiny (~KB) cross-core
reduction and final dot products happen on host in float64. No collectives.

Layout: d-major (partition = d) so every reduction above is a free-axis sum
riding on `accum_out` of ops we need anyway; PE does only 128x128 block
transposes of mu/logvar (identity matmuls into PSUM) and no reduction
matmuls at all. ACT fuses PSUM-evacuation of lvT with exp(+V); DVE fuses
w = muT*v with the W reduction, and A/B ride on the product passes.

Streaming: work is cut into half-units (b-block, i-half, d-chunk) whose
inputs are 1 MiB slab pairs + 512 KiB x quarters, issued interleaved so
complete half-units become ready uniformly across the whole DMA window —
the engines then pipeline directly behind the ~36 us DMA stream.
"""

import sys

import numpy as np

for _p in ("/opt/trn_rl_repo",):
    if _p not in sys.path:
        sys.path.append(_p)

B, D, H, W = 16, 512, 32, 32
HW = H * W
N = B * HW
NCORES = 8
BLKB = B // NCORES          # b-blocks per core (2)
ROWS = N // NCORES          # rows per core (2048)
NT = ROWS // 128            # 128-row i-tiles per core (16)
NDC = D // 128              # d chunks (4)
SLAB = 4                    # i-tiles per mu/lv DMA slab (= 512 i)
NU = BLKB * NDC             # full units per core (8)
HHW = HW // 2               # i-extent of a half-unit (512)

_prog_cache = {}


def build_program():
    import concourse.bacc as bacc
    import concourse.tile as tile
    from concourse import mybir

    f32 = mybir.dt.float32
    AF = mybir.ActivationFunctionType
    OP = mybir.AluOpType
    AX = mybir.AxisListType

    nc = bacc.Bacc(
        "TRN2",
        target_bir_lowering=False,
        debug=False,
        enable_asserts=False,
        num_devices=NCORES,
    )

    x_d = nc.dram_tensor("x_s", (BLKB, D, HW), f32, kind="ExternalInput").ap()
    mu_d = nc.dram_tensor("mu_s", (ROWS, D), f32, kind="ExternalInput").ap()
    lv_d = nc.dram_tensor("lv_s", (ROWS, D), f32, kind="ExternalInput").ap()
    id_d = nc.dram_tensor("ident", (128, 128), f32, kind="ExternalInput").ap()

    # o_misc columns (partition p -> d = 128*dc+p), unit u = b*NDC+dc:
    #   [0,2NU)   V   (col 2u+h)
    #   [2NU,4NU) W   (col 2NU+2u+h)
    #   [4NU,5NU) S1  (col 4NU+u)
    #   [5NU,6NU) S2  (col 5NU+u)
    #   [6NU,7NU) A   (col 6NU+u)
    #   [7NU,8NU) B   (col 7NU+u)
    o_misc = nc.dram_tensor("o_misc", (128, 8 * NU), f32, kind="ExternalOutput").ap()

    with tile.TileContext(nc) as tc:
        with (
            tc.tile_pool(name="const", bufs=1) as constp,
            tc.tile_pool(name="xnat", bufs=1) as xp,
            tc.tile_pool(name="slab", bufs=2) as slp,
            tc.tile_pool(name="vw", bufs=5) as vwp,
            tc.tile_pool(name="pp", bufs=5) as ppool,
            tc.tile_pool(name="scr", bufs=3) as scrp,
            tc.tile_pool(name="accum", bufs=1) as accp,
            tc.tile_pool(name="psum", bufs=4, space="PSUM") as pp,
        ):
            ident = constp.tile([128, 128], f32)
            nc.sync.dma_start(ident[:], id_d[:])

            acc = [
                accp.tile([128, w_], f32, tag=f"acc{q}", name=f"acc{q}")
                for q, w_ in enumerate((2 * NU, 2 * NU, NU, NU, NU, NU))
            ]

            def load_slab_pair(sidx):
                rows = lv_d[128 * SLAB * sidx : 128 * SLAB * (sidx + 1), :]
                lv_sl = slp.tile([128, SLAB * D], f32, tag="lv_sl", name="lv_sl")
                nc.gpsimd.dma_start(
                    lv_sl[:], rows.rearrange("(g p) f -> p g f", p=128)
                )
                rows = mu_d[128 * SLAB * sidx : 128 * SLAB * (sidx + 1), :]
                mu_sl = slp.tile([128, SLAB * D], f32, tag="mu_sl", name="mu_sl")
                nc.gpsimd.dma_start(
                    mu_sl[:], rows.rearrange("(g p) f -> p g f", p=128)
                )
                return mu_sl, lv_sl

            def load_x_quarter(b, dc):
                t_ = xp.tile([128, HW], f32, tag=f"x_{b}_{dc}", name=f"x_{b}_{dc}")
                nc.sync.dma_start(t_[:], x_d[b, 128 * dc : 128 * (dc + 1), :])
                return t_

            xq = {}
            vhold = {}
            whold = {}
            phold = {}

            # prologue loads: slab pair (0,0) then x quarters of b0
            slab = load_slab_pair(0)
            for dc in range(NDC):
                xq[(0, dc)] = load_x_quarter(0, dc)

            for b in range(BLKB):
                for h in range(2):
                    if (b, h) != (0, 0):
                        slab = load_slab_pair(2 * b + h)
                    if h == 1 and b + 1 < BLKB:
                        # prefetch next block's x quarters mid-stream
                        for dc in range(NDC):
                            xq[(b + 1, dc)] = load_x_quarter(b + 1, dc)
                    mu_sl, lv_sl = slab

                    for dc in range(NDC):
                        u = b * NDC + dc
                        xs = xq[(b, dc)][:]

                        lvT = pp.tile([128, HHW], f32, tag="lvT", name="lvT")
                        muT = pp.tile([128, HHW], f32, tag="muT", name="muT")
                        for blk in range(4):
                            col = D * blk + 128 * dc
                            for dst, src in ((lvT, lv_sl), (muT, mu_sl)):
                                nc.tensor.matmul(
                                    dst[:, 128 * blk : 128 * (blk + 1)],
                                    src[:, col : col + 128],
                                    ident[:],
                                    is_transpose=True,
                                    start=(blk == 0),
                                    stop=(blk == 3),
                                )

                        if h == 0:
                            vhold[dc] = vwp.tile([128, HW], f32, tag="v", name="v_u")
                            whold[dc] = vwp.tile([128, HW], f32, tag="w", name="w_u")
                        v_u = vhold[dc]
                        w_u = whold[dc]

                        nc.scalar.activation(
                            v_u[:, HHW * h : HHW * (h + 1)], lvT[:],
                            AF.Exp, scale=-1.0,
                            accum_out=acc[0][:, 2 * u + h : 2 * u + h + 1],
                        )
                        nc.vector.scalar_tensor_tensor(
                            out=w_u[:, HHW * h : HHW * (h + 1)], in0=muT[:],
                            scalar=1.0, in1=v_u[:, HHW * h : HHW * (h + 1)],
                            op0=OP.mult, op1=OP.mult,
                            accum_out=acc[1][:, 2 * u + h : 2 * u + h + 1],
                        )

                        if h == 0:
                            # x-only passes fire on the first half so they
                            # overlap the wait for the second slab pair
                            p_u = ppool.tile([128, HW], f32, tag="p", name="p_u")
                            phold[dc] = p_u
                            nc.scalar.activation(
                                p_u[:], xs, AF.Square,
                                accum_out=acc[3][:, u : u + 1],
                            )
                            if dc % 2 == 0:
                                s1scr = scrp.tile(
                                    [128, HW], f32, tag="s1scr", name="s1scr"
                                )
                                nc.scalar.activation(
                                    s1scr[:], xs, AF.Copy,
                                    accum_out=acc[2][:, u : u + 1],
                                )
                            else:
                                nc.vector.tensor_reduce(
                                    acc[2][:, u : u + 1], xs, AX.X, OP.add
                                )
                        else:
                            a_scr = scrp.tile([128, HW], f32, tag="a", name="a_scr")
                            nc.vector.scalar_tensor_tensor(
                                out=a_scr[:], in0=phold[dc][:], scalar=1.0,
                                in1=v_u[:], op0=OP.mult, op1=OP.mult,
                                accum_out=acc[4][:, u : u + 1],
                            )
                            b_scr = scrp.tile([128, HW], f32, tag="b", name="b_scr")
                            nc.vector.scalar_tensor_tensor(
                                out=b_scr[:], in0=w_u[:], scalar=1.0, in1=xs,
                                op0=OP.mult, op1=OP.mult,
                                accum_out=acc[5][:, u : u + 1],
                            )

            off = 0
            for q in range(6):
                w_ = acc[q].shape[1]
                nc.sync.dma_start(o_misc[:, off : off + w_], acc[q][:])
                off += w_

    nc.compile()
    return nc


def get_program():
    if "nc" not in _prog_cache:
        _prog_cache["nc"] = build_program()
    return _prog_cache["nc"]


def make_in_maps(x, p_mu, p_logvar):
    x = np.ascontiguousarray(np.asarray(x, dtype=np.float32)).reshape(B, D, HW)
    p_mu = np.ascontiguousarray(np.asarray(p_mu, dtype=np.float32))
    p_logvar = np.ascontiguousarray(np.asarray(p_logvar, dtype=np.float32))
    in_maps = []
    for c in range(NCORES):
        in_maps.append(
            {
                "x_s": np.ascontiguousarray(x[BLKB * c : BLKB * (c + 1)]),
                "mu_s": np.ascontiguousarray(p_mu[ROWS * c : ROWS * (c + 1)]),
                "lv_s": np.ascontiguousarray(p_logvar[ROWS * c : ROWS * (c + 1)]),
                "ident": np.eye(128, dtype=np.float32),
            }
        )
    return in_maps


def finish_host(results):
    """Combine per-core partials (float64) into the scalar loss."""
    Vv = np.zeros(D)
    Ww = np.zeros(D)
    S2 = np.zeros(D)
    S1 = np.zeros(D)
    A = 0.0
    Bb = 0.0
    for r in results:
        misc = r["o_misc"].astype(np.float64)
        for u in range(NU):
            b, dc = divmod(u, NDC)
            dsl = slice(128 * dc, 128 * (dc + 1))
            Vv[dsl] += misc[:, 2 * u] + misc[:, 2 * u + 1]
            Ww[dsl] += misc[:, 2 * NU + 2 * u] + misc[:, 2 * NU + 2 * u + 1]
            S1[dsl] += misc[:, 4 * NU + u]
            S2[dsl] += misc[:, 5 * NU + u]
            A += float(misc[:, 6 * NU + u].sum())
            Bb += float(misc[:, 7 * NU + u].sum())
    m1 = S1 / N
    m2 = S2 / N
    S = A - 2.0 * Bb - float(np.dot(m2, Vv)) + 2.0 * float(np.dot(m1, Ww))
    return np.float32(-0.5 / N * S)


def run_on_device(x, p_mu, p_logvar, trace=False, **kw):
    from concourse import bass_utils

    nc = get_program()
    in_maps = make_in_maps(x, p_mu, p_logvar)
    return bass_utils.run_bass_kernel_spmd(
        nc, in_maps, list(range(NCORES)), trace=trace, **kw
    )


def kernel(x, p_mu, p_logvar):
    res = run_on_device(x, p_mu, p_logvar)
    return finish_host(res.results)


# revision 37
# speedup vs baseline: 1.1350x; 1.0241x over previous
"""CLUB loss kernel for Trainium2, 8-core data-parallel SPMD.

Math: with flat_x (N,D) [from x (B,D,H,W) -> (B*H*W, D)], v = exp(-p_logvar),
  loss = mean_i[ -0.5*sum_d ((x-mu)^2 - (m2 - 2*mu*m1 + mu^2)) * v ]
       = (-0.5/N) * [ A - 2B - dot(m2, V) + 2*dot(m1, W) ]
where
  A  = sum_{i,d} x^2 v          B  = sum_{i,d} x mu v
  V_d = sum_i v                 W_d = sum_i mu v
  m1 = S1/N, m2 = S2/N,  S1_d = sum_i x,  S2_d = sum_i x^2
All terms are per-core-local partial sums; the tiny (~KB) cross-core
reduction and final dot products happen on host in float64. No collectives.

Layout: d-major (partition = d) so every reduction above is a free-axis sum
riding on `accum_out` of ops we need anyway; PE does only 128x128 block
transposes of mu/logvar (identity matmuls into PSUM) and no reduction
matmuls. Engine split: ACT = exp (fused with lvT PSUM evacuation + V), x^2
(+S2), copy (+S1) — the x-only passes run early; DVE = the three product
passes w/a/b with their W/A/B reductions fused, kept per-512-wide half so
the post-last-DMA tail stays short.

Streaming: 512 KiB mu/lv slabs + 512 KiB x quarters issued interleaved so
complete (b-block, i-half, d-chunk) work becomes ready uniformly across the
~36 us DMA window and the engines pipeline directly behind the stream.
"""

import sys

import numpy as np

for _p in ("/opt/trn_rl_repo",):
    if _p not in sys.path:
        sys.path.append(_p)

B, D, H, W = 16, 512, 32, 32
HW = H * W
N = B * HW
NCORES = 8
BLKB = B // NCORES          # b-blocks per core (2)
ROWS = N // NCORES          # rows per core (2048)
NT = ROWS // 128            # 128-row i-tiles per core (16)
NDC = D // 128              # d chunks (4)
SLAB = 2                    # i-tiles per mu/lv DMA slab (= 256 i, 512 KiB)
NU = BLKB * NDC             # full units per core (8)
HHW = HW // 2               # i-extent of a half-unit (512)

_prog_cache = {}


def build_program():
    import concourse.bacc as bacc
    import concourse.tile as tile
    from concourse import mybir

    f32 = mybir.dt.float32
    AF = mybir.ActivationFunctionType
    OP = mybir.AluOpType

    nc = bacc.Bacc(
        "TRN2",
        target_bir_lowering=False,
        debug=False,
        enable_asserts=False,
        num_devices=NCORES,
    )

    x_d = nc.dram_tensor("x_s", (BLKB, D, HW), f32, kind="ExternalInput").ap()
    mu_d = nc.dram_tensor("mu_s", (ROWS, D), f32, kind="ExternalInput").ap()
    lv_d = nc.dram_tensor("lv_s", (ROWS, D), f32, kind="ExternalInput").ap()
    id_d = nc.dram_tensor("ident", (128, 128), f32, kind="ExternalInput").ap()

    # o_misc columns (partition p -> d = 128*dc+p), unit u = b*NDC+dc,
    # half-col hc = 2*u+h:
    #   [0,2NU) V | [2NU,4NU) W | [4NU,6NU) A | [6NU,8NU) B   (per half)
    #   [8NU,9NU) S1 | [9NU,10NU) S2                          (per unit)
    o_misc = nc.dram_tensor("o_misc", (128, 10 * NU), f32, kind="ExternalOutput").ap()

    with tile.TileContext(nc) as tc:
        with (
            tc.tile_pool(name="const", bufs=1) as constp,
            tc.tile_pool(name="xnat", bufs=1) as xp,
            tc.tile_pool(name="slab", bufs=5) as slp,
            tc.tile_pool(name="vw", bufs=4) as vwp,
            tc.tile_pool(name="ppool", bufs=5) as ppool,
            tc.tile_pool(name="scr", bufs=3) as scrp,
            tc.tile_pool(name="accum", bufs=1) as accp,
            tc.tile_pool(name="psum", bufs=4, space="PSUM") as pp,
        ):
            ident = constp.tile([128, 128], f32)
            nc.sync.dma_start(ident[:], id_d[:])

            acc = [
                accp.tile([128, w_], f32, tag=f"acc{q}", name=f"acc{q}")
                for q, w_ in enumerate((2 * NU, 2 * NU, 2 * NU, 2 * NU, NU, NU))
            ]

            lv_slabs = {}
            mu_slabs = {}
            xq = {}

            def load_slab(dram, store, sidx, tag):
                rows = dram[128 * SLAB * sidx : 128 * SLAB * (sidx + 1), :]
                t_ = slp.tile([128, SLAB * D], f32, tag=tag, name=tag)
                nc.gpsimd.dma_start(t_[:], rows.rearrange("(g p) f -> p g f", p=128))
                store[sidx] = t_

            def load_x_quarter(b, dc):
                t_ = xp.tile([128, HW], f32, tag=f"x_{b}_{dc}", name=f"x_{b}_{dc}")
                nc.sync.dma_start(t_[:], x_d[b, 128 * dc : 128 * (dc + 1), :])
                xq[(b, dc)] = t_

            def load_half_block(b, h):
                # slabs covering i-tiles [8b+4h, 8b+4h+4) = 2 slabs per tensor
                s0 = (8 * b + 4 * h) // SLAB
                for s in (s0, s0 + 1):
                    load_slab(lv_d, lv_slabs, s, "lv_sl")
                for s in (s0, s0 + 1):
                    load_slab(mu_d, mu_slabs, s, "mu_sl")

            # interleaved issue order for uniform readiness
            load_half_block(0, 0)
            for dc in range(NDC):
                load_x_quarter(0, dc)
            load_half_block(0, 1)

            phold = {}
            for b in range(BLKB):
                for h in range(2):
                    if b > 0 and h == 0:
                        load_half_block(b, 0)
                        for dc in range(NDC):
                            load_x_quarter(b, dc)
                    if b > 0 and h == 1:
                        load_half_block(b, 1)

                    for dc in range(NDC):
                        u = b * NDC + dc
                        hc = 2 * u + h
                        xs = xq[(b, dc)][:, HHW * h : HHW * (h + 1)]

                        lvT = pp.tile([128, HHW], f32, tag="lvT", name="lvT")
                        muT = pp.tile([128, HHW], f32, tag="muT", name="muT")
                        for blk in range(4):
                            t_i = 8 * b + 4 * h + blk
                            sl_col = D * (t_i % SLAB) + 128 * dc
                            for dst, store in ((lvT, lv_slabs), (muT, mu_slabs)):
                                nc.tensor.matmul(
                                    dst[:, 128 * blk : 128 * (blk + 1)],
                                    store[t_i // SLAB][:, sl_col : sl_col + 128],
                                    ident[:],
                                    is_transpose=True,
                                    start=(blk == 0),
                                    stop=(blk == 3),
                                )

                        v_u = vwp.tile([128, HHW], f32, tag="v", name="v_u")
                        nc.scalar.activation(
                            v_u[:], lvT[:], AF.Exp, scale=-1.0,
                            accum_out=acc[0][:, hc : hc + 1],
                        )
                        w_u = vwp.tile([128, HHW], f32, tag="w", name="w_u")
                        nc.vector.scalar_tensor_tensor(
                            out=w_u[:], in0=muT[:], scalar=1.0, in1=v_u[:],
                            op0=OP.mult, op1=OP.mult,
                            accum_out=acc[1][:, hc : hc + 1],
                        )

                        if h == 0:
                            # x-only ACT passes: full unit width, fire early
                            xfull = xq[(b, dc)][:]
                            p_u = ppool.tile([128, HW], f32, tag="p", name="p_u")
                            phold[dc] = p_u
                            nc.scalar.activation(
                                p_u[:], xfull, AF.Square,
                                accum_out=acc[5][:, u : u + 1],
                            )
                            s1scr = scrp.tile(
                                [128, HW], f32, tag="s1scr", name="s1scr"
                            )
                            nc.scalar.activation(
                                s1scr[:], xfull, AF.Copy,
                                accum_out=acc[4][:, u : u + 1],
                            )

                        ph = phold[dc][:, HHW * h : HHW * (h + 1)]
                        a_scr = scrp.tile([128, HHW], f32, tag="a", name="a_scr")
                        nc.vector.scalar_tensor_tensor(
                            out=a_scr[:], in0=ph, scalar=1.0, in1=v_u[:],
                            op0=OP.mult, op1=OP.mult,
                            accum_out=acc[2][:, hc : hc + 1],
                        )
                        b_scr = scrp.tile([128, HHW], f32, tag="b", name="b_scr")
                        nc.vector.scalar_tensor_tensor(
                            out=b_scr[:], in0=w_u[:], scalar=1.0, in1=xs,
                            op0=OP.mult, op1=OP.mult,
                            accum_out=acc[3][:, hc : hc + 1],
                        )

            off = 0
            for q in (0, 1, 2, 3, 4, 5):
                w_ = acc[q].shape[1]
                nc.sync.dma_start(o_misc[:, off : off + w_], acc[q][:])
                off += w_

    nc.compile()
    return nc


def get_program():
    if "nc" not in _prog_cache:
        _prog_cache["nc"] = build_program()
    return _prog_cache["nc"]


def make_in_maps(x, p_mu, p_logvar):
    x = np.ascontiguousarray(np.asarray(x, dtype=np.float32)).reshape(B, D, HW)
    p_mu = np.ascontiguousarray(np.asarray(p_mu, dtype=np.float32))
    p_logvar = np.ascontiguousarray(np.asarray(p_logvar, dtype=np.float32))
    in_maps = []
    for c in range(NCORES):
        in_maps.append(
            {
                "x_s": np.ascontiguousarray(x[BLKB * c : BLKB * (c + 1)]),
                "mu_s": np.ascontiguousarray(p_mu[ROWS * c : ROWS * (c + 1)]),
                "lv_s": np.ascontiguousarray(p_logvar[ROWS * c : ROWS * (c + 1)]),
                "ident": np.eye(128, dtype=np.float32),
            }
        )
    return in_maps


def finish_host(results):
    """Combine per-core partials (float64) into the scalar loss."""
    Vv = np.zeros(D)
    Ww = np.zeros(D)
    S2 = np.zeros(D)
    S1 = np.zeros(D)
    A = 0.0
    Bb = 0.0
    for r in results:
        misc = r["o_misc"].astype(np.float64)
        for u in range(NU):
            b, dc = divmod(u, NDC)
            dsl = slice(128 * dc, 128 * (dc + 1))
            for h in range(2):
                hc = 2 * u + h
                Vv[dsl] += misc[:, hc]
                Ww[dsl] += misc[:, 2 * NU + hc]
                A += float(misc[:, 4 * NU + hc].sum())
                Bb += float(misc[:, 6 * NU + hc].sum())
            S1[dsl] += misc[:, 8 * NU + u]
            S2[dsl] += misc[:, 9 * NU + u]
    m1 = S1 / N
    m2 = S2 / N
    S = A - 2.0 * Bb - float(np.dot(m2, Vv)) + 2.0 * float(np.dot(m1, Ww))
    return np.float32(-0.5 / N * S)


def run_on_device(x, p_mu, p_logvar, trace=False, **kw):
    from concourse import bass_utils

    nc = get_program()
    in_maps = make_in_maps(x, p_mu, p_logvar)
    return bass_utils.run_bass_kernel_spmd(
        nc, in_maps, list(range(NCORES)), trace=trace, **kw
    )


def kernel(x, p_mu, p_logvar):
    res = run_on_device(x, p_mu, p_logvar)
    return finish_host(res.results)
